# revision 59
# baseline (speedup 1.0000x reference)
"""CameraHead Trainium2 kernel — data-parallel over b*v across 8 NeuronCores.

v2: layer-major fp8 DoubleRow edition. Per core, activations live in four
static [128, 4x8192] feature-major SBUF buffers (X, HA, HB, HC); each of the
six 512x512 Linears runs as a full 8192-token pass (DR fp8, K=256/pass) before
the next layer starts, so every drain/elementwise op has a whole layer of PE
cover. The block-2 residual is de-fused: x2 = x0 + h3a is a cheap elementwise
add (DVE+GPSIMD) instead of a second accumulated matmul stream, cutting PE
work by 1/7. Pooling: Sum(h3a) via deferred 4D tensor_reduce on DVE;
Sum(h3b) comes free from per-sample accum_out on the l5 drains (h3b itself is
never materialized — drains write a dead scratch). Tail: bf16 MLP/head
matmuls + a lean d-tracking Jacobi for the SVD->SO(3) projection.
"""
import sys
import numpy as np

sys.path.insert(0, '/opt/trn_rl_repo')

import ml_dtypes  # noqa: E402

import concourse.bacc as bacc  # noqa: E402
import concourse.mybir as mybir  # noqa: E402
from concourse import tile  # noqa: E402
from concourse import dve_ops as _dvo  # noqa: E402
from concourse.bass_utils import run_bass_kernel_spmd  # noqa: E402
from concourse.dve_spec import (  # noqa: E402
    C0, C1, C2, One, Spec, Src0, Src1, select as dve_select, sq as dve_sq,
)


def _reg_op(name, body, ref):
    """Register a custom DVE op (per-NEFF uop table; no firmware change).

    The uops sha pin is bootstrapped by parsing compile()'s drift error."""
    for op in _dvo.OPS:
        if op.name == name:
            return op
    import re as _re

    from concourse.dve_table_gen import dve_ver_for

    row = _dvo._CUSTOM_DVE_ROW_BASE + len(_dvo.OPS)
    assert row < 0x20, "custom DVE opcode rows exhausted"
    spec = Spec(body=body, reference=ref)
    op = _dvo.DveOp(name, spec, subdim=False, uops_sha={})
    _dvo.OPS.append(op)
    _dvo._SUB_OPCODE_FOR_NAME[name] = row
    _dvo.CUSTOM_DVE_SPECS[name] = spec
    ver = dve_ver_for("TRN2")
    try:
        op.compile(ver)
    except ValueError as e:
        m = _re.search(r'uops_sha\["' + ver + r'"\]="([0-9a-f]+)"', str(e))
        if not m:
            raise
        op.uops_sha[ver] = m.group(1)
        op.compile(ver)
    return op


_f32 = np.float32
OP_AXPBY = _reg_op(
    "ANT_AXPBY", Src0 * C0 + Src1 * C1,
    lambda in0, in1, s0, s1, imm2: (in0 * s0 + in1 * s1).astype(_f32))
OP_AXMBY = _reg_op(
    "ANT_AXMBY", Src0 * C0 - Src1 * C1,
    lambda in0, in1, s0, s1, imm2: (in0 * s0 - in1 * s1).astype(_f32))
OP_SQDIFF = _reg_op(
    "ANT_SQDIFF", dve_sq(Src0) - dve_sq(Src1),
    lambda in0, in1, s0, s1, imm2: (in0 * in0 - in1 * in1).astype(_f32))


def _xy2_body():
    t = Src0 * Src1
    return t + t


OP_XY2 = _reg_op(
    "ANT_XY2", _xy2_body(),
    lambda in0, in1, s0, s1, imm2: (2.0 * in0 * in1).astype(_f32))
OP_WHERE = _reg_op(
    "ANT_WHERE", dve_select(C0, Src0, Src1),
    lambda in0, in1, s0, s1, imm2: np.where(
        s0 != 0, in0, in1).astype(_f32))
OP_WHERENEG = _reg_op(
    "ANT_WHERENEG", dve_select(C0, -Src0, Src1),
    lambda in0, in1, s0, s1, imm2: np.where(
        s0 != 0, -in0, in1).astype(_f32))
OP_SQSUM = _reg_op(
    "ANT_SQSUM", dve_sq(Src0) + dve_sq(Src1),
    lambda in0, in1, s0, s1, imm2: (in0 * in0 + in1 * in1).astype(_f32))


def _xyw_body():
    t = Src0 * Src1
    return (t + t) * C0


# 2 * ch * sh * w2 — raw sine
OP_XYW2 = _reg_op(
    "ANT_XYW2", _xyw_body(),
    lambda in0, in1, s0, s1, imm2: (2.0 * in0 * in1 * s0).astype(_f32))
# (ch^2 - sh^2) * w2  — raw cosine
OP_CSUBW = _reg_op(
    "ANT_CSUBW", (dve_sq(Src0) - dve_sq(Src1)) * C0,
    lambda in0, in1, s0, s1, imm2: (
        (in0 * in0 - in1 * in1) * s0).astype(_f32))
# select(g < 0, imm2, raw)
OP_SELPOS = _reg_op(
    "ANT_SELPOS", dve_select(Src1 < C1, C2, Src0),
    lambda in0, in1, s0, s1, imm2: np.where(
        in1 < s1, imm2, in0).astype(_f32))


def _axpb4y_body():
    t = Src1 * C1
    u = t + t
    return Src0 * C0 + (u + u)


# d' = c2*d + 4*s2*pt
OP_AXPB4Y = _reg_op(
    "ANT_AXPB4Y", _axpb4y_body(),
    lambda in0, in1, s0, s1, imm2: (in0 * s0 + 4.0 * in1 * s1).astype(_f32))
# pt' = c2*pt - 0.25*s2*d   (imm2 carries the 0.25)
OP_AXMBYC = _reg_op(
    "ANT_AXMBYC", Src0 * C0 - (Src1 * C1) * C2,
    lambda in0, in1, s0, s1, imm2: (in0 * s0 - in1 * s1 * imm2).astype(_f32))

F32 = mybir.dt.float32
BF16 = mybir.dt.bfloat16
F16 = mybir.dt.float16
F8 = mybir.dt.float8e4
AF = mybir.ActivationFunctionType
ALU = mybir.AluOpType
AX = mybir.AxisListType
DR = mybir.MatmulPerfMode.DoubleRow

N_CORES = 8
D = 512
SAMPLES = 256          # b*v
TOK = 256              # tokens per sample
S_CORE = SAMPLES // N_CORES       # 32 samples per core
T_CORE = S_CORE * TOK             # 8192 token rows per core
T_SUP = 1024           # supertile: 4 samples, drained per layer pass
N_SUP = T_CORE // T_SUP           # 8
S_SUP = T_SUP // TOK              # 4 samples per supertile

N_ROT = 8              # lean Jacobi rotations
GPS_X2 = True          # GPSIMD handles 2 of the 4 x2 chunks

GAMMA = float(3.0 + 2.0 * np.sqrt(2.0))
CQ45 = float(np.cos(np.pi / 4))


# ---------------------------------------------------------------------------
# small-op emitter for the SVD tail: SSA-style column allocation on a scratch
# tile; every value is an AP (or list of APs).
# ---------------------------------------------------------------------------
class Emit:
    def __init__(self, nc, pool):
        self.nc = nc
        self.scr = pool.tile([32, 2048], F32, tag="svd_scratch",
                             name="svd_scratch")
        self.ptr = 0

    def new(self, n=1):
        c = self.ptr
        self.ptr += n
        assert self.ptr <= 2048, "svd scratch overflow"
        return self.scr[:, c:c + n]

    def tt(self, op, a, b, n=1):
        o = self.new(n)
        self.nc.vector.tensor_tensor(o, a, b, op)
        return o

    def tt3(self, op, a, b, n=9):
        o = self.new(n)
        self.nc.vector.tensor_tensor(
            o.rearrange("p (i j) -> p i j", i=3, j=n // 3), a, b, op)
        return o

    def ts(self, op, a, s, n=1):
        o = self.new(n)
        self.nc.vector.tensor_scalar(o, a, s, None, op)
        return o

    def stt(self, a, scal, b, op0, op1, n=1):
        o = self.new(n)
        self.nc.vector.scalar_tensor_tensor(o, a, scal, b, op0=op0, op1=op1)
        return o

    # --- gpsimd variant (tensor_tensor only; Pool supports no Ptr ops) ---
    def gtt(self, op, a, b, n=1):
        o = self.new(n)
        self.nc.gpsimd.tensor_tensor(o, a, b, op)
        return o

    def rsqrt(self, a, n=1):
        t = self.new(n)
        self.nc.scalar.activation(t, a, AF.Sqrt)
        o = self.new(n)
        self.nc.vector.reciprocal(o, t)
        return o

    def cdve(self, op, in0, in1, s0=0.0, s1=0.0, imm2=0.0, n=1, out=None):
        if out is None:
            out = self.new(n)
        self.nc.vector._custom_dve(op, out=out, in0=in0, in1=in1,
                                   s0=s0, s1=s1, imm2=imm2)
        return out

    def const(self, val, n=1):
        o = self.new(n)
        self.nc.vector.memset(o, val)
        return o


def _bcast_r(ap3):
    return ap3.unsqueeze(2).broadcast_to([32, 3, 3])


def _bcast_l(ap3):
    return ap3.unsqueeze(1).broadcast_to([32, 3, 3])


def emit_svd_so3(nc, em, m_ap, pose_tile):
    """m_ap: [32,9] raw 3x3 per sample (row-major). Writes the SO(3)
    projection into pose_tile columns (4r+c for r,c in 0..2).

    Lean d-tracking Jacobi: state is (d01,d12,d02) eigenvalue differences and
    (p01,p12,p02) halved off-diagonals; 15 DVE ops per rotation."""
    # --- row normalize ---
    sq = em.tt(ALU.mult, m_ap, m_ap, 9)
    t = em.tt(ALU.add, sq[:, 0:9:3], sq[:, 1:9:3], 3)
    r2 = em.tt(ALU.add, t, sq[:, 2:9:3], 3)
    r2c = em.ts(ALU.max, r2, 1e-24, 3)
    rinv = em.rsqrt(r2c, 3)
    A = em.tt3(ALU.mult, m_ap.rearrange("p (r c) -> p r c", r=3, c=3),
               _bcast_r(rinv), 9)

    # --- S = A^T A (s_ij at col 3i+j) ---
    terms = []
    for r in range(3):
        arow = A[:, 3 * r:3 * r + 3]
        terms.append(em.tt3(ALU.mult, _bcast_r(arow), _bcast_l(arow), 9))
    s01 = em.tt(ALU.add, terms[0], terms[1], 9)
    S9 = em.tt(ALU.add, s01, terms[2], 9)

    # d/pt state (SSA-tracked APs)
    d01 = em.tt(ALU.subtract, S9[:, 0:1], S9[:, 4:5])
    d12 = em.tt(ALU.subtract, S9[:, 4:5], S9[:, 8:9])
    d02 = em.tt(ALU.add, d01, d12)
    p01 = em.ts(ALU.mult, S9[:, 1:2], 0.5)
    p12 = em.ts(ALU.mult, S9[:, 5:6], 0.5)
    p02 = em.ts(ALU.mult, S9[:, 2:3], 0.5)

    # V columns as [32,3] blocks, init = identity
    Vc = []
    for j in range(3):
        vj = em.new(3)
        nc.vector.memset(vj, 0.0)
        nc.vector.memset(vj[:, j:j + 1], 1.0)
        Vc.append(vj)

    st = {'d01': d01, 'd12': d12, 'd02': d02,
          'p01': p01, 'p12': p12, 'p02': p02}

    def angle(ch, sh):
        # gate condition gamma*sh^2 >= ch^2 is exactly craw <= cos(pi/4),
        # so the pi/4 fallback is a clamp on craw and a craw-keyed select.
        ssum = em.cdve(OP_SQSUM, ch, sh)
        w2 = em.new(1)
        nc.vector.reciprocal(w2, ssum)
        craw = em.cdve(OP_CSUBW, ch, sh, s0=w2)
        sraw = em.cdve(OP_XYW2, ch, sh, s0=w2)
        c = em.ts(ALU.max, craw, CQ45)
        s = em.cdve(OP_SELPOS, sraw, craw, s1=CQ45, imm2=CQ45)
        c2 = em.cdve(OP_SQDIFF, c, s)
        s2 = em.cdve(OP_XY2, c, s)
        return c, s, c2, s2

    def np_pair(pa, pb_, c, s):
        """Rotate the r-row p~ pair (feeds the next rotation's angle; fused
        custom ops on DVE beat a 6-op GPSIMD chain plus handoff)."""
        na = em.cdve(OP_AXPBY, pa, pb_, s0=c, s1=s)
        nb = em.cdve(OP_AXMBY, pb_, pa, s0=c, s1=s)
        return na, nb

    def vup(p, q, c, s):
        # V rotation runs on GPSIMD, off the DVE critical chain (the V
        # columns are only consumed after the rotation loop).
        cb = c.broadcast_to([32, 3])
        sb = s.broadcast_to([32, 3])
        t1 = em.gtt(ALU.mult, Vc[q], sb, 3)
        t2 = em.gtt(ALU.mult, Vc[p], cb, 3)
        nvp = em.gtt(ALU.add, t2, t1, 3)
        t3 = em.gtt(ALU.mult, Vc[p], sb, 3)
        t4 = em.gtt(ALU.mult, Vc[q], cb, 3)
        nvq = em.gtt(ALU.subtract, t4, t3, 3)
        Vc[p], Vc[q] = nvp, nvq

    for k in range(N_ROT):
        rt = k % 3
        if rt == 0:     # (p,q,r) = (0,1,2)
            c, s, c2, s2 = angle(st['d01'], st['p01'])
            dd = em.cdve(OP_AXPB4Y, st['d01'], st['p01'], s0=c2, s1=s2)
            pn = em.cdve(OP_AXMBYC, st['p01'], st['d01'], s0=c2, s1=s2,
                         imm2=0.25)
            t_ = em.tt(ALU.add, st['d02'], st['d12'])
            nd02 = em.cdve(OP_AXPBY, t_, dd, s0=0.5, s1=0.5)
            nd12 = em.cdve(OP_AXMBY, t_, dd, s0=0.5, s1=0.5)
            np02, np12 = np_pair(st['p02'], st['p12'], c, s)
            st.update(d01=dd, p01=pn, d02=nd02, d12=nd12, p02=np02, p12=np12)
            vup(0, 1, c, s)
        elif rt == 1:   # (1,2,0)
            c, s, c2, s2 = angle(st['d12'], st['p12'])
            dd = em.cdve(OP_AXPB4Y, st['d12'], st['p12'], s0=c2, s1=s2)
            pn = em.cdve(OP_AXMBYC, st['p12'], st['d12'], s0=c2, s1=s2,
                         imm2=0.25)
            t_ = em.tt(ALU.add, st['d01'], st['d02'])
            nd01 = em.cdve(OP_AXMBY, t_, dd, s0=0.5, s1=0.5)
            nd02 = em.cdve(OP_AXPBY, t_, dd, s0=0.5, s1=0.5)
            np01, np02 = np_pair(st['p01'], st['p02'], c, s)
            st.update(d12=dd, p12=pn, d01=nd01, d02=nd02, p01=np01, p02=np02)
            vup(1, 2, c, s)
        else:           # (0,2,1)
            c, s, c2, s2 = angle(st['d02'], st['p02'])
            dd = em.cdve(OP_AXPB4Y, st['d02'], st['p02'], s0=c2, s1=s2)
            pn = em.cdve(OP_AXMBYC, st['p02'], st['d02'], s0=c2, s1=s2,
                         imm2=0.25)
            t_ = em.tt(ALU.subtract, st['d01'], st['d12'])
            nd01 = em.cdve(OP_AXPBY, t_, dd, s0=0.5, s1=0.5)
            nd12 = em.cdve(OP_AXMBY, dd, t_, s0=0.5, s1=0.5)
            np01, np12 = np_pair(st['p01'], st['p12'], c, s)
            st.update(d02=dd, p02=pn, d01=nd01, d12=nd12, p01=np01, p12=np12)
            vup(0, 2, c, s)

    # --- sort eigenpairs descending (det(V) stays +1 via column negation) ---
    def cond_swap(i, j):
        key_ij = f'd{i}{j}'
        k = 3 - i - j
        key_ik = f'd{min(i,k)}{max(i,k)}'
        key_jk = f'd{min(j,k)}{max(j,k)}'
        mask = em.ts(ALU.is_lt, st[key_ij], 0.0)
        nij = em.cdve(OP_WHERENEG, st[key_ij], st[key_ij], s0=mask)
        nik = em.cdve(OP_WHERE, st[key_jk], st[key_ik], s0=mask)
        njk = em.cdve(OP_WHERE, st[key_ik], st[key_jk], s0=mask)
        st[key_ij], st[key_ik], st[key_jk] = nij, nik, njk
        # V swap on GPSIMD: vi' = vi + m*(vj-vi); vj' = vj - m*(vi+vj)
        m3 = mask.broadcast_to([32, 3])
        dv = em.gtt(ALU.subtract, Vc[j], Vc[i], 3)
        md = em.gtt(ALU.mult, dv, m3, 3)
        vi = em.gtt(ALU.add, Vc[i], md, 3)
        sv = em.gtt(ALU.add, Vc[i], Vc[j], 3)
        ms = em.gtt(ALU.mult, sv, m3, 3)
        vj = em.gtt(ALU.subtract, Vc[j], ms, 3)
        Vc[i], Vc[j] = vi, vj

    # only slot 2 (the smallest eigenvalue) must be in place: the det-fix
    # lands on it via u3 = u1 x u2 and v3 = v1 x v2; slots 0/1 order and
    # signs cancel (det(V) stays +1 through the rotation-style swaps)
    cond_swap(1, 2)
    cond_swap(0, 2)

    # --- B columns (j=0,1): b_j[r] = sum_c A[r][c] * V[c][j] ---
    Astr = [A[:, c:c + 7:3] for c in range(3)]

    def bcol(j):
        t0 = em.cdve(OP_AXPBY, Astr[0], Astr[1],
                     s0=Vc[j][:, 0:1], s1=Vc[j][:, 1:2], n=3)
        return em.stt(Astr[2], Vc[j][:, 2:3], t0, ALU.mult, ALU.add, 3)

    def bcol_gps(j):
        t0 = em.gtt(ALU.mult, Astr[0],
                    Vc[j][:, 0:1].broadcast_to([32, 3]), 3)
        t1 = em.gtt(ALU.mult, Astr[1],
                    Vc[j][:, 1:2].broadcast_to([32, 3]), 3)
        t01 = em.gtt(ALU.add, t0, t1, 3)
        t2 = em.gtt(ALU.mult, Astr[2],
                    Vc[j][:, 2:3].broadcast_to([32, 3]), 3)
        return em.gtt(ALU.add, t01, t2, 3)

    b0 = bcol(0)
    b1 = bcol_gps(1)

    def normalize(v3):
        sqv = em.tt(ALU.mult, v3, v3, 3)
        n_ = em.tt(ALU.add, sqv[:, 0:1], sqv[:, 1:2])
        n_ = em.tt(ALU.add, n_, sqv[:, 2:3])
        nc_ = em.ts(ALU.max, n_, 1e-30)
        inv = em.rsqrt(nc_)
        return em.ts(ALU.mult, v3, inv, 3)

    u1 = normalize(b0)
    p_ = em.tt(ALU.mult, u1, b1, 3)
    d_ = em.tt(ALU.add, p_[:, 0:1], p_[:, 1:2])
    d_ = em.tt(ALU.add, d_, p_[:, 2:3])
    dneg = em.ts(ALU.mult, d_, -1.0)
    b2o = em.stt(u1, dneg, b1, ALU.mult, ALU.add, 3)
    u2 = normalize(b2o)
    u3 = em.new(3)
    for k, (i1, i2) in enumerate(((1, 2), (2, 0), (0, 1))):
        em.cdve(OP_AXMBY, u1[:, i1:i1 + 1], u1[:, i2:i2 + 1],
                s0=u2[:, i2:i2 + 1], s1=u2[:, i1:i1 + 1],
                out=u3[:, k:k + 1])

    # --- R = u1 v1^T + u2 v2^T + u3 v3^T ---
    t0 = em.tt3(ALU.mult, _bcast_r(u1), _bcast_l(Vc[0]), 9)
    t1 = em.tt3(ALU.mult, _bcast_r(u2), _bcast_l(Vc[1]), 9)
    t01 = em.tt(ALU.add, t0, t1, 9)
    t2 = em.tt3(ALU.mult, _bcast_r(u3), _bcast_l(Vc[2]), 9)
    pose_R = pose_tile[:].rearrange("p (r c) -> p r c", r=4, c=4)[:, 0:3, 0:3]
    nc.vector.tensor_tensor(
        pose_R, t01.rearrange("p (r c) -> p r c", r=3, c=3),
        t2.rearrange("p (r c) -> p r c", r=3, c=3), ALU.add)


# ---------------------------------------------------------------------------
# kernel build
# ---------------------------------------------------------------------------
def build_nc():
    nc = bacc.Bacc("TRN2", target_bir_lowering=False)

    # quarter-major x layout: [quarter, feature, token-in-quarter] so every
    # (chunk, quarter) DMA is one fully contiguous 256KB block
    xT8 = nc.dram_tensor("xT8", [4, D, 2048], F8, kind="ExternalInput")
    w8 = nc.dram_tensor("w8", [6, 128, 2048], F8, kind="ExternalInput")
    bs = nc.dram_tensor("bs", [6, D], F32, kind="ExternalInput")
    x0s = nc.dram_tensor("x0s", [128, 4 * S_CORE], F32, kind="ExternalInput")
    mwt = nc.dram_tensor("mwt", [2, D, D], BF16, kind="ExternalInput")
    mbs = nc.dram_tensor("mbs", [2, D], F32, kind="ExternalInput")
    hwT = nc.dram_tensor("hwT", [D, 12], BF16, kind="ExternalInput")
    hb = nc.dram_tensor("hb", [S_CORE, 12], F32, kind="ExternalInput")
    pose = nc.dram_tensor("pose", [S_CORE, 16], F32, kind="ExternalOutput")

    with tile.TileContext(nc) as tc:
        with (
            tc.tile_pool(name="wp", bufs=1) as wpool,
            tc.tile_pool(name="ps", bufs=4, space="PSUM") as pspool,
            tc.tile_pool(name="sm", bufs=1) as smpool,
        ):
            # warm both ACT tables (Sqrt for the SVD tail, Relu for drains)
            # while the first DMAs stream
            warm = smpool.tile([32, 1], F32, tag="warm", name="warm")
            nc.vector.memset(warm[:], 0.0)
            nc.scalar.activation(warm[:], warm[:], AF.Sqrt)
            nc.scalar.activation(warm[:], warm[:], AF.Relu)

            # ---- static activation buffers (feature-major: chunk k holds
            # features 128k..128k+127 for all 8192 tokens) ----
            X = wpool.tile([128, 4 * T_CORE], F8, tag="X", name="X")
            HA = wpool.tile([128, 4 * T_CORE], F8, tag="HA", name="HA")
            HB = wpool.tile([128, 4 * T_CORE], F8, tag="HB", name="HB")
            HC = wpool.tile([128, 4 * T_CORE], F8, tag="HC", name="HC")

            w_sb = [wpool.tile([128, 2048], F8, tag=f"w{l}", name=f"w{l}")
                    for l in range(6)]
            b_sb = wpool.tile([128, 24], F32, tag="b", name="b_sb")

            # X in chunk-quarters (the l0/ti(2q,2q+1) matmuls need quarter q
            # of every chunk), interleaved with the layer-0 weights so the
            # first matmuls fire a few us in and never starve after that
            def xdma(eng, c, q):
                eng.dma_start(
                    X[:, T_CORE * c + 2048 * q:T_CORE * c + 2048 * (q + 1)],
                    xT8[q, 128 * c:128 * (c + 1), :])

            def xdma8(eng, c, e):
                eng.dma_start(
                    X[:, T_CORE * c + 1024 * e:T_CORE * c + 1024 * (e + 1)],
                    xT8[0, 128 * c:128 * (c + 1),
                        1024 * e:1024 * (e + 1)])

            # first supertile as eighths so the very first matmuls fire ASAP
            xdma8(nc.gpsimd, 0, 0)
            xdma8(nc.scalar, 1, 0)
            nc.sync.dma_start(w_sb[0][:, 0:512], w8[0, :, 0:512])
            xdma8(nc.gpsimd, 2, 0)
            xdma8(nc.scalar, 3, 0)
            xdma8(nc.gpsimd, 0, 1)
            xdma8(nc.scalar, 1, 1)
            nc.sync.dma_start(w_sb[0][:, 512:2048], w8[0, :, 512:2048])
            xdma8(nc.gpsimd, 2, 1)
            xdma8(nc.scalar, 3, 1)
            # biases immediately behind w0: the first l0 drains need them
            for l in range(6):
                nc.sync.dma_start(b_sb[:, 4 * l:4 * l + 4],
                                  bs[l].rearrange("(o p) -> p o", p=128, o=4))
            # q1 rides partly on the sync queue (w1/w2 aren't needed for
            # ~30/60us) so the l0 stream never starves
            xdma(nc.sync, 0, 1)
            xdma(nc.scalar, 1, 1)
            xdma(nc.sync, 2, 1)
            xdma(nc.scalar, 3, 1)
            for q in range(2, 4):
                xdma(nc.gpsimd, 0, q)
                xdma(nc.gpsimd, 2, q)
            nc.sync.dma_start(w_sb[1][:], w8[1])
            nc.sync.dma_start(w_sb[2][:], w8[2])
            nc.sync.dma_start(w_sb[3][:], w8[3])
            nc.sync.dma_start(w_sb[5][:], w8[5])

            x0s_sb = wpool.tile([128, 4 * S_CORE], F32, tag="x0s",
                                name="x0s_sb")
            nc.sync.dma_start(x0s_sb[:], x0s[:])
            mw_sb = [wpool.tile([128, 2048], BF16, tag=f"mw{l}",
                                name=f"mw{l}") for l in range(2)]
            for l in range(2):
                for k in range(4):
                    nc.sync.dma_start(
                        mw_sb[l][:, D * k:D * (k + 1)],
                        mwt[l, 128 * k:128 * (k + 1), :])
            mb_sb = wpool.tile([128, 8], F32, tag="mb", name="mb_sb")
            for l in range(2):
                nc.sync.dma_start(mb_sb[:, 4 * l:4 * l + 4],
                                  mbs[l].rearrange("(o p) -> p o", p=128, o=4))
            hw_sb = wpool.tile([128, 48], BF16, tag="hw", name="hw_sb")
            for k in range(4):
                nc.sync.dma_start(hw_sb[:, 12 * k:12 * (k + 1)],
                                  hwT[128 * k:128 * (k + 1), :])
            hb_sb = wpool.tile([32, 12], F32, tag="hbt", name="hb_sb")
            nc.sync.dma_start(hb_sb[:], hb[:])

            # pooling accumulators: pb1 = per-sample Sum(h3a), pb2 =
            # per-sample Sum(h3b); both via DVE tensor_reduce (pb1 deferred
            # per-supertile, pb2 fine-grained right behind the l5 drains)
            pb1 = wpool.tile([128, 4 * S_CORE], F16, tag="pb1", name="pb1")
            pb2 = wpool.tile([128, 4 * S_CORE], F16, tag="pb2", name="pb2")
            # pooling tree scratch: tensor_reduce is stuck at 1 elem/cycle on
            # DVE, but 2-byte tensor_tensor hits the 2x packed mode - so sum
            # 256->128->64 with TT adds, then one small reduce
            tr1a = [wpool.tile([128, 2048], BF16, tag=f"tr1a{i}",
                               name=f"tr1a{i}") for i in range(2)]
            tr2a = wpool.tile([128, 1024], BF16, tag="tr2a", name="tr2a")
            tr1b = [wpool.tile([128, 2048], BF16, tag=f"tr1b{i}",
                               name=f"tr1b{i}") for i in range(2)]
            tr2b = [wpool.tile([128, 1024], BF16, tag=f"tr2b{i}",
                               name=f"tr2b{i}") for i in range(2)]

            def wap(l, o, kp):
                c0 = (o * 2 + kp) * 256
                return w_sb[l][:, c0:c0 + 256].rearrange(
                    "p (i m) -> p i m", i=2)

            def rhs(src, kp, ti, th):
                v = src[:].rearrange("p (k t) -> p k t", k=4)
                t0 = 1024 * ti + 512 * th
                return v[:, 2 * kp:2 * kp + 2, t0:t0 + 512]

            def hslice(dst, o, ti):
                c0 = T_CORE * o + 1024 * ti
                return dst[:, c0:c0 + 1024]

            def _src_v(buf, ti):
                return buf[:].rearrange("p (k g t) -> p k g t", k=4,
                                        g=S_CORE)[:, :, 4 * ti:4 * ti + 4, :]

            def pool_tree1(ti):
                """pb1 tree: level 1 split GPSIMD/DVE, level 2 + reduce DVE.
                Lives in the l3 phase, where all drains go to ACT, so these
                are the only DVE ops and never block a drain."""
                v = _src_v(HC, ti)
                d1 = tr1a[ti % 2][:].rearrange("p (k s t) -> p k s t",
                                               k=4, s=4)
                nc.gpsimd.tensor_tensor(
                    d1[:, 0:2], v[:, 0:2, :, 0:128], v[:, 0:2, :, 128:256],
                    ALU.add)
                nc.vector.tensor_tensor(
                    d1[:, 2:4], v[:, 2:4, :, 0:128], v[:, 2:4, :, 128:256],
                    ALU.add)
                t1 = tr1a[ti % 2][:].rearrange("p (g h t) -> p g h t",
                                               g=16, h=2)
                d2 = tr2a[:].rearrange("p (g t) -> p g t", g=16)
                nc.vector.tensor_tensor(d2, t1[:, :, 0, :], t1[:, :, 1, :],
                                        ALU.add)
                dst = pb1[:].rearrange("p (k s) -> p k s",
                                       k=4)[:, :, 4 * ti:4 * ti + 4]
                with nc.allow_low_precision("fp16 pooling partials"):
                    nc.vector.tensor_reduce(dst, d2, axis=AX.X, op=ALU.add)

            def pool_tree2(ti):
                """pb2 tree: level 1 split GPSIMD/DVE, level 2 + reduce DVE.
                Ping-pong scratch so consecutive supertiles' trees overlap
                (matters most for ti6/ti7 right at the end)."""
                v = _src_v(HB, ti)
                w1, w2 = tr1b[ti % 2], tr2b[ti % 2]
                d1 = w1[:].rearrange("p (k s t) -> p k s t", k=4, s=4)
                nc.gpsimd.tensor_tensor(
                    d1[:, 0:2], v[:, 0:2, :, 0:128], v[:, 0:2, :, 128:256],
                    ALU.add)
                nc.vector.tensor_tensor(
                    d1[:, 2:4], v[:, 2:4, :, 0:128], v[:, 2:4, :, 128:256],
                    ALU.add)
                t1 = w1[:].rearrange("p (g h t) -> p g h t", g=16, h=2)
                d2 = w2[:].rearrange("p (g t) -> p g t", g=16)
                nc.vector.tensor_tensor(d2, t1[:, :, 0, :], t1[:, :, 1, :],
                                        ALU.add)
                dst = pb2[:].rearrange("p (k s) -> p k s",
                                       k=4)[:, :, 4 * ti:4 * ti + 4]
                with nc.allow_low_precision("fp16 pooling partials"):
                    nc.vector.tensor_reduce(dst, d2, axis=AX.X, op=ALU.add)

            def drain(l, o, ti, ps, dst, eng):
                h = hslice(dst, o, ti)
                bias = b_sb[:, 4 * l + o:4 * l + o + 1]
                if eng == 'act':
                    nc.scalar.activation(h, ps[:], AF.Relu, bias=bias,
                                         scale=1.0)
                else:
                    nc.vector.tensor_scalar(h, ps[:], bias, 0.0,
                                            ALU.add, ALU.max)

            def x2_emit(ti):
                """x2 = x0 + h3a (fp8), chunk k: GPSIMD takes 0/1, DVE 2/3."""
                for k in (3, 2, 0, 1):
                    c0 = T_CORE * k + 1024 * ti
                    dst = HA[:, c0:c0 + 1024]
                    eng = nc.gpsimd if (GPS_X2 and k < 2) else nc.vector
                    eng.tensor_tensor(dst, HC[:, c0:c0 + 1024],
                                      X[:, c0:c0 + 1024], ALU.add)

            # ACT drains o0-o2, DVE o3 — except l3, where ACT takes all 4 so
            # the DVE queue holds only pb1 tree ops (no drain ever queues
            # behind a pooling op there)
            def eng_for(l, o, ti):
                if l == 3:
                    return 'act'
                if l == 5 and ti >= 6:
                    # last pair: DVE drains chunks 2+3 so its pb2 tree ops
                    # never wait on ACT's queue right at the end
                    return 'dve' if o >= 2 else 'act'
                return 'dve' if o == 3 else 'act'

            # buffer roles per layer: src, dst
            ROLES = [(X, HA), (HA, HB), (HB, HC), (HA, HB), (HB, X),
                     (X, HB)]

            # l4/l5 interleave in ti-pairs so h3b (and its pooling reduces)
            # starts ~40us before the end of the matmul stream
            pool_f32 = smpool.tile([128, 4 * S_CORE], F32, tag="poolf",
                                   name="pool_f32")
            pool_bf = smpool.tile([128, 4 * S_CORE], BF16, tag="poolb",
                                  name="pool_bf")

            PHASES = [(0, range(8)), (1, range(8)), (2, range(8)),
                      (3, range(8))]
            for p in range(4):
                PHASES.append((4, range(2 * p, 2 * p + 2)))
                PHASES.append((5, range(2 * p, 2 * p + 2)))

            # pb1 pooling placement: one tree per ti in the l3 phase
            PB1_AT = {(3, ti): ti for ti in range(N_SUP)}

            for pi, (l, tis) in enumerate(PHASES):
                src, dst = ROLES[l]
                for ti in tis:
                    for o in range(4):
                        pst = pspool.tile([128, 1024], F32, tag="ps",
                                          name="ps")
                        mi = 0
                        for kp in range(2):
                            for th in range(2):
                                nc.tensor.matmul(
                                    pst[:, 512 * th:512 * (th + 1)],
                                    wap(l, o, kp), rhs(src, kp, ti, th),
                                    start=(mi < 2), stop=(mi >= 2),
                                    perf_mode=DR)
                                mi += 1
                        drain(l, o, ti, pst, dst, eng_for(l, o, ti))
                    # scalar-queue X/w4 issues ride between early l0 drains
                    # (issuing them all in the preamble delays the first
                    # drains by ~6us of queue-issue time)
                    if l == 0 and ti in (1, 3, 5):
                        if ti == 5:
                            nc.scalar.dma_start(w_sb[4][:], w8[4])
                        else:
                            q = 2 if ti == 1 else 3
                            xdma(nc.scalar, 1, q)
                            xdma(nc.scalar, 3, q)
                    if l == 5:
                        pool_tree2(ti)
                    if l == 2:
                        x2_emit(ti)
                    t1_ = PB1_AT.get((pi, ti))
                    if t1_ is not None:
                        pool_tree1(t1_)
                    if pi == 11 and ti == 6:
                        # pb1 is long complete: pre-add x0s on idle GPSIMD so
                        # the tail needs only one combine op after pb2
                        nc.gpsimd.tensor_tensor(pool_f32[:], pb1[:],
                                                x0s_sb[:], ALU.add)

            # ---- pooled = (x0s + pb1) + pb2: single combine on DVE,
            # in-FIFO right behind the final reduce ----
            nc.vector.tensor_tensor(pool_bf[:], pool_f32[:], pb2[:],
                                    ALU.add)

            # ---- tail MLPs (bf16), psum reused from the main pool ----
            f_prev = pool_bf
            scales = [1.0 / TOK, 1.0]
            for l in range(2):
                f_out = smpool.tile([128, 4 * S_CORE], BF16, tag=f"f{l}",
                                    name=f"f{l}")
                for o in range(4):
                    ps_w = pspool.tile([128, 1024], F32, tag="ps",
                                       name="pst")
                    psm = ps_w[:, 0:S_CORE]
                    for k in range(4):
                        nc.tensor.matmul(
                            psm,
                            mw_sb[l][:, D * k + 128 * o:D * k + 128 * (o + 1)],
                            f_prev[:, S_CORE * k:S_CORE * (k + 1)],
                            start=(k == 0), stop=(k == 3))
                    nc.scalar.activation(
                        f_out[:, S_CORE * o:S_CORE * (o + 1)], psm, AF.Relu,
                        bias=mb_sb[:, 4 * l + o:4 * l + o + 1],
                        scale=scales[l])
                f_prev = f_out

            # ---- heads: [32 samples, 12] = t(3) ++ rot(9) ----
            psh_w = pspool.tile([128, 1024], F32, tag="ps", name="psh")
            psh = psh_w[0:32, 0:12]
            for k in range(4):
                nc.tensor.matmul(psh,
                                 f_prev[:, S_CORE * k:S_CORE * (k + 1)],
                                 hw_sb[:, 12 * k:12 * (k + 1)],
                                 start=(k == 0), stop=(k == 3))
            mm = smpool.tile([32, 12], F32, tag="mm", name="mm")
            nc.vector.tensor_add(mm[:], psh, hb_sb[:])

            # ---- pose assembly + SVD ----
            pose_t = smpool.tile([32, 16], F32, tag="pose", name="pose_t")
            nc.vector.memset(pose_t[:], 0.0)
            nc.vector.memset(pose_t[:, 15:16], 1.0)
            nc.vector.tensor_copy(
                pose_t[:].rearrange("p (r c) -> p r c", r=4, c=4)[:, 0:3, 3],
                mm[:, 0:3])

            em = Emit(nc, smpool)
            emit_svd_so3(nc, em, mm[:, 3:12], pose_t)

            nc.sync.dma_start(pose[:], pose_t[:])

    nc.compile()
    return nc


_NC_CACHE = None


def _get_nc():
    global _NC_CACHE
    if _NC_CACHE is None:
        _NC_CACHE = build_nc()
    return _NC_CACHE


F8NP = ml_dtypes.float8_e4m3fn
BF16NP = ml_dtypes.bfloat16


def kernel(**inputs):
    feat = np.asarray(inputs["feat"], dtype=np.float32)
    b_, v_, n_, d_ = feat.shape
    xs = feat.reshape(b_ * v_, n_, d_)
    x0sum = xs.sum(axis=1, dtype=np.float32)          # (256, 512)

    # DoubleRow weight prepack: [p, o, kp, i, m] <- wT[128*(2kp+i)+p, 128o+m]
    w8_list = []
    for blk in (1, 2):
        for li in (1, 2, 3):
            wT = np.asarray(inputs[f"r{blk}_w{li}"], np.float32).T
            arr = wT.astype(F8NP).reshape(2, 2, 128, 4, 128)
            arr = np.ascontiguousarray(arr.transpose(2, 3, 0, 1, 4))
            w8_list.append(arr.reshape(128, 2048))
    w8 = np.stack(w8_list)
    bs = np.stack([np.asarray(inputs[f"r{blk}_b{li}"], np.float32)
                   for blk in (1, 2) for li in (1, 2, 3)])
    mwt = np.stack([np.ascontiguousarray(
        np.asarray(inputs[f"m_w{li}"], np.float32).T).astype(BF16NP)
        for li in (1, 2)])
    mbs = np.stack([np.asarray(inputs[f"m_b{li}"], np.float32)
                    for li in (1, 2)])
    hwT = np.ascontiguousarray(np.concatenate(
        [np.asarray(inputs["t_w"], np.float32).T,
         np.asarray(inputs["rot_w"], np.float32).T], axis=1)).astype(BF16NP)
    hb = np.broadcast_to(np.concatenate(
        [np.asarray(inputs["t_b"], np.float32),
         np.asarray(inputs["rot_b"], np.float32)])[None, :],
        (S_CORE, 12)).copy()

    in_maps = []
    for c in range(N_CORES):
        xT = xs[c * S_CORE:(c + 1) * S_CORE].reshape(T_CORE, D).T  # (512, T)
        xT8 = np.ascontiguousarray(
            xT.reshape(D, 4, 2048).transpose(1, 0, 2)).astype(F8NP)
        xs_c = x0sum[c * S_CORE:(c + 1) * S_CORE]     # (32, 512)
        x0s = np.ascontiguousarray(
            xs_c.T.reshape(4, 128, S_CORE).transpose(1, 0, 2).reshape(
                128, 4 * S_CORE))
        in_maps.append({
            "xT8": xT8, "w8": w8, "bs": bs, "x0s": x0s, "mwt": mwt,
            "mbs": mbs, "hwT": hwT, "hb": hb,
        })

    nc = _get_nc()
    import os
    kwargs = {}
    if os.environ.get("KERNEL_TRACE") == "1":
        kwargs["trace"] = True
    res = run_bass_kernel_spmd(nc, in_maps, core_ids=list(range(N_CORES)),
                               **kwargs)
    if kwargs.get("trace"):
        kernel.last_results = res
    poses = np.concatenate([r["pose"] for r in res.results], axis=0)
    return poses.reshape(b_, v_, 4, 4)


# revision 61
# speedup vs baseline: 1.0063x; 1.0063x over previous
"""CameraHead Trainium2 kernel — data-parallel over b*v across 8 NeuronCores.

v2: layer-major fp8 DoubleRow edition. Per core, activations live in four
static [128, 4x8192] feature-major SBUF buffers (X, HA, HB, HC); each of the
six 512x512 Linears runs as a full 8192-token pass (DR fp8, K=256/pass) before
the next layer starts, so every drain/elementwise op has a whole layer of PE
cover. The block-2 residual is de-fused: x2 = x0 + h3a is a cheap elementwise
add (DVE+GPSIMD) instead of a second accumulated matmul stream, cutting PE
work by 1/7. Pooling: Sum(h3a) via deferred 4D tensor_reduce on DVE;
Sum(h3b) comes free from per-sample accum_out on the l5 drains (h3b itself is
never materialized — drains write a dead scratch). Tail: bf16 MLP/head
matmuls + a lean d-tracking Jacobi for the SVD->SO(3) projection.
"""
import sys
import numpy as np

sys.path.insert(0, '/opt/trn_rl_repo')

import ml_dtypes  # noqa: E402

import concourse.bacc as bacc  # noqa: E402
import concourse.mybir as mybir  # noqa: E402
from concourse import tile  # noqa: E402
from concourse import dve_ops as _dvo  # noqa: E402
from concourse.bass_utils import run_bass_kernel_spmd  # noqa: E402
from concourse.dve_spec import (  # noqa: E402
    C0, C1, C2, One, Spec, Src0, Src1, select as dve_select, sq as dve_sq,
)


def _reg_op(name, body, ref):
    """Register a custom DVE op (per-NEFF uop table; no firmware change).

    The uops sha pin is bootstrapped by parsing compile()'s drift error."""
    for op in _dvo.OPS:
        if op.name == name:
            return op
    import re as _re

    from concourse.dve_table_gen import dve_ver_for

    row = _dvo._CUSTOM_DVE_ROW_BASE + len(_dvo.OPS)
    assert row < 0x20, "custom DVE opcode rows exhausted"
    spec = Spec(body=body, reference=ref)
    op = _dvo.DveOp(name, spec, subdim=False, uops_sha={})
    _dvo.OPS.append(op)
    _dvo._SUB_OPCODE_FOR_NAME[name] = row
    _dvo.CUSTOM_DVE_SPECS[name] = spec
    ver = dve_ver_for("TRN2")
    try:
        op.compile(ver)
    except ValueError as e:
        m = _re.search(r'uops_sha\["' + ver + r'"\]="([0-9a-f]+)"', str(e))
        if not m:
            raise
        op.uops_sha[ver] = m.group(1)
        op.compile(ver)
    return op


_f32 = np.float32
OP_AXPBY = _reg_op(
    "ANT_AXPBY", Src0 * C0 + Src1 * C1,
    lambda in0, in1, s0, s1, imm2: (in0 * s0 + in1 * s1).astype(_f32))
OP_AXMBY = _reg_op(
    "ANT_AXMBY", Src0 * C0 - Src1 * C1,
    lambda in0, in1, s0, s1, imm2: (in0 * s0 - in1 * s1).astype(_f32))
OP_SQDIFF = _reg_op(
    "ANT_SQDIFF", dve_sq(Src0) - dve_sq(Src1),
    lambda in0, in1, s0, s1, imm2: (in0 * in0 - in1 * in1).astype(_f32))


def _xy2_body():
    t = Src0 * Src1
    return t + t


OP_XY2 = _reg_op(
    "ANT_XY2", _xy2_body(),
    lambda in0, in1, s0, s1, imm2: (2.0 * in0 * in1).astype(_f32))
OP_WHERE = _reg_op(
    "ANT_WHERE", dve_select(C0, Src0, Src1),
    lambda in0, in1, s0, s1, imm2: np.where(
        s0 != 0, in0, in1).astype(_f32))
OP_WHERENEG = _reg_op(
    "ANT_WHERENEG", dve_select(C0, -Src0, Src1),
    lambda in0, in1, s0, s1, imm2: np.where(
        s0 != 0, -in0, in1).astype(_f32))
OP_SQSUM = _reg_op(
    "ANT_SQSUM", dve_sq(Src0) + dve_sq(Src1),
    lambda in0, in1, s0, s1, imm2: (in0 * in0 + in1 * in1).astype(_f32))


def _xyw_body():
    t = Src0 * Src1
    return (t + t) * C0


# 2 * ch * sh * w2 — raw sine
OP_XYW2 = _reg_op(
    "ANT_XYW2", _xyw_body(),
    lambda in0, in1, s0, s1, imm2: (2.0 * in0 * in1 * s0).astype(_f32))
# (ch^2 - sh^2) * w2  — raw cosine
OP_CSUBW = _reg_op(
    "ANT_CSUBW", (dve_sq(Src0) - dve_sq(Src1)) * C0,
    lambda in0, in1, s0, s1, imm2: (
        (in0 * in0 - in1 * in1) * s0).astype(_f32))
# select(g < 0, imm2, raw)
OP_SELPOS = _reg_op(
    "ANT_SELPOS", dve_select(Src1 < C1, C2, Src0),
    lambda in0, in1, s0, s1, imm2: np.where(
        in1 < s1, imm2, in0).astype(_f32))


def _axpb4y_body():
    t = Src1 * C1
    u = t + t
    return Src0 * C0 + (u + u)


# d' = c2*d + 4*s2*pt
OP_AXPB4Y = _reg_op(
    "ANT_AXPB4Y", _axpb4y_body(),
    lambda in0, in1, s0, s1, imm2: (in0 * s0 + 4.0 * in1 * s1).astype(_f32))
# pt' = c2*pt - 0.25*s2*d   (imm2 carries the 0.25)
OP_AXMBYC = _reg_op(
    "ANT_AXMBYC", Src0 * C0 - (Src1 * C1) * C2,
    lambda in0, in1, s0, s1, imm2: (in0 * s0 - in1 * s1 * imm2).astype(_f32))

F32 = mybir.dt.float32
BF16 = mybir.dt.bfloat16
F16 = mybir.dt.float16
F8 = mybir.dt.float8e4
AF = mybir.ActivationFunctionType
ALU = mybir.AluOpType
AX = mybir.AxisListType
DR = mybir.MatmulPerfMode.DoubleRow

N_CORES = 8
D = 512
SAMPLES = 256          # b*v
TOK = 256              # tokens per sample
S_CORE = SAMPLES // N_CORES       # 32 samples per core
T_CORE = S_CORE * TOK             # 8192 token rows per core
T_SUP = 1024           # supertile: 4 samples, drained per layer pass
N_SUP = T_CORE // T_SUP           # 8
S_SUP = T_SUP // TOK              # 4 samples per supertile

N_ROT = 8              # lean Jacobi rotations
GPS_X2 = True          # GPSIMD handles 2 of the 4 x2 chunks

GAMMA = float(3.0 + 2.0 * np.sqrt(2.0))
CQ45 = float(np.cos(np.pi / 4))


# ---------------------------------------------------------------------------
# small-op emitter for the SVD tail: SSA-style column allocation on a scratch
# tile; every value is an AP (or list of APs).
# ---------------------------------------------------------------------------
class Emit:
    def __init__(self, nc, pool):
        self.nc = nc
        self.scr = pool.tile([32, 2048], F32, tag="svd_scratch",
                             name="svd_scratch")
        self.ptr = 0

    def new(self, n=1):
        c = self.ptr
        self.ptr += n
        assert self.ptr <= 2048, "svd scratch overflow"
        return self.scr[:, c:c + n]

    def tt(self, op, a, b, n=1):
        o = self.new(n)
        self.nc.vector.tensor_tensor(o, a, b, op)
        return o

    def tt3(self, op, a, b, n=9):
        o = self.new(n)
        self.nc.vector.tensor_tensor(
            o.rearrange("p (i j) -> p i j", i=3, j=n // 3), a, b, op)
        return o

    def ts(self, op, a, s, n=1):
        o = self.new(n)
        self.nc.vector.tensor_scalar(o, a, s, None, op)
        return o

    def stt(self, a, scal, b, op0, op1, n=1):
        o = self.new(n)
        self.nc.vector.scalar_tensor_tensor(o, a, scal, b, op0=op0, op1=op1)
        return o

    # --- gpsimd variant (tensor_tensor only; Pool supports no Ptr ops) ---
    def gtt(self, op, a, b, n=1):
        o = self.new(n)
        self.nc.gpsimd.tensor_tensor(o, a, b, op)
        return o

    def rsqrt(self, a, n=1):
        t = self.new(n)
        self.nc.scalar.activation(t, a, AF.Sqrt)
        o = self.new(n)
        self.nc.vector.reciprocal(o, t)
        return o

    def cdve(self, op, in0, in1, s0=0.0, s1=0.0, imm2=0.0, n=1, out=None):
        if out is None:
            out = self.new(n)
        self.nc.vector._custom_dve(op, out=out, in0=in0, in1=in1,
                                   s0=s0, s1=s1, imm2=imm2)
        return out

    def const(self, val, n=1):
        o = self.new(n)
        self.nc.vector.memset(o, val)
        return o


def _bcast_r(ap3):
    return ap3.unsqueeze(2).broadcast_to([32, 3, 3])


def _bcast_l(ap3):
    return ap3.unsqueeze(1).broadcast_to([32, 3, 3])


def emit_svd_so3(nc, em, m_ap, pose_tile):
    """m_ap: [32,9] raw 3x3 per sample (row-major). Writes the SO(3)
    projection into pose_tile columns (4r+c for r,c in 0..2).

    Lean d-tracking Jacobi: state is (d01,d12,d02) eigenvalue differences and
    (p01,p12,p02) halved off-diagonals; 15 DVE ops per rotation."""
    # --- row normalize ---
    sq = em.tt(ALU.mult, m_ap, m_ap, 9)
    t = em.tt(ALU.add, sq[:, 0:9:3], sq[:, 1:9:3], 3)
    r2 = em.tt(ALU.add, t, sq[:, 2:9:3], 3)
    r2c = em.ts(ALU.max, r2, 1e-24, 3)
    rinv = em.rsqrt(r2c, 3)
    A = em.tt3(ALU.mult, m_ap.rearrange("p (r c) -> p r c", r=3, c=3),
               _bcast_r(rinv), 9)

    # --- S = A^T A (s_ij at col 3i+j) ---
    terms = []
    for r in range(3):
        arow = A[:, 3 * r:3 * r + 3]
        terms.append(em.tt3(ALU.mult, _bcast_r(arow), _bcast_l(arow), 9))
    s01 = em.tt(ALU.add, terms[0], terms[1], 9)
    S9 = em.tt(ALU.add, s01, terms[2], 9)

    # d/pt state (SSA-tracked APs)
    d01 = em.tt(ALU.subtract, S9[:, 0:1], S9[:, 4:5])
    d12 = em.tt(ALU.subtract, S9[:, 4:5], S9[:, 8:9])
    d02 = em.tt(ALU.add, d01, d12)
    p01 = em.ts(ALU.mult, S9[:, 1:2], 0.5)
    p12 = em.ts(ALU.mult, S9[:, 5:6], 0.5)
    p02 = em.ts(ALU.mult, S9[:, 2:3], 0.5)

    # V columns as [32,3] blocks, init = identity
    Vc = []
    for j in range(3):
        vj = em.new(3)
        nc.vector.memset(vj, 0.0)
        nc.vector.memset(vj[:, j:j + 1], 1.0)
        Vc.append(vj)

    st = {'d01': d01, 'd12': d12, 'd02': d02,
          'p01': p01, 'p12': p12, 'p02': p02}

    def angle(ch, sh):
        # gate condition gamma*sh^2 >= ch^2 is exactly craw <= cos(pi/4),
        # so the pi/4 fallback is a clamp on craw and a craw-keyed select.
        ssum = em.cdve(OP_SQSUM, ch, sh)
        w2 = em.new(1)
        nc.vector.reciprocal(w2, ssum)
        craw = em.cdve(OP_CSUBW, ch, sh, s0=w2)
        sraw = em.cdve(OP_XYW2, ch, sh, s0=w2)
        c = em.ts(ALU.max, craw, CQ45)
        s = em.cdve(OP_SELPOS, sraw, craw, s1=CQ45, imm2=CQ45)
        c2 = em.cdve(OP_SQDIFF, c, s)
        s2 = em.cdve(OP_XY2, c, s)
        return c, s, c2, s2

    def np_pair(pa, pb_, c, s):
        """Rotate the r-row p~ pair (feeds the next rotation's angle; fused
        custom ops on DVE beat a 6-op GPSIMD chain plus handoff)."""
        na = em.cdve(OP_AXPBY, pa, pb_, s0=c, s1=s)
        nb = em.cdve(OP_AXMBY, pb_, pa, s0=c, s1=s)
        return na, nb

    def vup(p, q, c, s):
        # V rotation runs on GPSIMD, off the DVE critical chain (the V
        # columns are only consumed after the rotation loop).
        cb = c.broadcast_to([32, 3])
        sb = s.broadcast_to([32, 3])
        t1 = em.gtt(ALU.mult, Vc[q], sb, 3)
        t2 = em.gtt(ALU.mult, Vc[p], cb, 3)
        nvp = em.gtt(ALU.add, t2, t1, 3)
        t3 = em.gtt(ALU.mult, Vc[p], sb, 3)
        t4 = em.gtt(ALU.mult, Vc[q], cb, 3)
        nvq = em.gtt(ALU.subtract, t4, t3, 3)
        Vc[p], Vc[q] = nvp, nvq

    for k in range(N_ROT):
        rt = k % 3
        if rt == 0:     # (p,q,r) = (0,1,2)
            c, s, c2, s2 = angle(st['d01'], st['p01'])
            dd = em.cdve(OP_AXPB4Y, st['d01'], st['p01'], s0=c2, s1=s2)
            pn = em.cdve(OP_AXMBYC, st['p01'], st['d01'], s0=c2, s1=s2,
                         imm2=0.25)
            t_ = em.tt(ALU.add, st['d02'], st['d12'])
            nd02 = em.cdve(OP_AXPBY, t_, dd, s0=0.5, s1=0.5)
            nd12 = em.cdve(OP_AXMBY, t_, dd, s0=0.5, s1=0.5)
            np02, np12 = np_pair(st['p02'], st['p12'], c, s)
            st.update(d01=dd, p01=pn, d02=nd02, d12=nd12, p02=np02, p12=np12)
            vup(0, 1, c, s)
        elif rt == 1:   # (1,2,0)
            c, s, c2, s2 = angle(st['d12'], st['p12'])
            dd = em.cdve(OP_AXPB4Y, st['d12'], st['p12'], s0=c2, s1=s2)
            pn = em.cdve(OP_AXMBYC, st['p12'], st['d12'], s0=c2, s1=s2,
                         imm2=0.25)
            t_ = em.tt(ALU.add, st['d01'], st['d02'])
            nd01 = em.cdve(OP_AXMBY, t_, dd, s0=0.5, s1=0.5)
            nd02 = em.cdve(OP_AXPBY, t_, dd, s0=0.5, s1=0.5)
            np01, np02 = np_pair(st['p01'], st['p02'], c, s)
            st.update(d12=dd, p12=pn, d01=nd01, d02=nd02, p01=np01, p02=np02)
            vup(1, 2, c, s)
        else:           # (0,2,1)
            c, s, c2, s2 = angle(st['d02'], st['p02'])
            dd = em.cdve(OP_AXPB4Y, st['d02'], st['p02'], s0=c2, s1=s2)
            pn = em.cdve(OP_AXMBYC, st['p02'], st['d02'], s0=c2, s1=s2,
                         imm2=0.25)
            t_ = em.tt(ALU.subtract, st['d01'], st['d12'])
            nd01 = em.cdve(OP_AXPBY, t_, dd, s0=0.5, s1=0.5)
            nd12 = em.cdve(OP_AXMBY, dd, t_, s0=0.5, s1=0.5)
            np01, np12 = np_pair(st['p01'], st['p12'], c, s)
            st.update(d02=dd, p02=pn, d01=nd01, d12=nd12, p01=np01, p12=np12)
            vup(0, 2, c, s)

    # --- sort eigenpairs descending (det(V) stays +1 via column negation) ---
    def cond_swap(i, j):
        key_ij = f'd{i}{j}'
        k = 3 - i - j
        key_ik = f'd{min(i,k)}{max(i,k)}'
        key_jk = f'd{min(j,k)}{max(j,k)}'
        mask = em.ts(ALU.is_lt, st[key_ij], 0.0)
        nij = em.cdve(OP_WHERENEG, st[key_ij], st[key_ij], s0=mask)
        nik = em.cdve(OP_WHERE, st[key_jk], st[key_ik], s0=mask)
        njk = em.cdve(OP_WHERE, st[key_ik], st[key_jk], s0=mask)
        st[key_ij], st[key_ik], st[key_jk] = nij, nik, njk
        # V swap on GPSIMD: vi' = vi + m*(vj-vi); vj' = vj - m*(vi+vj)
        m3 = mask.broadcast_to([32, 3])
        dv = em.gtt(ALU.subtract, Vc[j], Vc[i], 3)
        md = em.gtt(ALU.mult, dv, m3, 3)
        vi = em.gtt(ALU.add, Vc[i], md, 3)
        sv = em.gtt(ALU.add, Vc[i], Vc[j], 3)
        ms = em.gtt(ALU.mult, sv, m3, 3)
        vj = em.gtt(ALU.subtract, Vc[j], ms, 3)
        Vc[i], Vc[j] = vi, vj

    # only slot 2 (the smallest eigenvalue) must be in place: the det-fix
    # lands on it via u3 = u1 x u2 and v3 = v1 x v2; slots 0/1 order and
    # signs cancel (det(V) stays +1 through the rotation-style swaps)
    cond_swap(1, 2)
    cond_swap(0, 2)

    # --- B columns (j=0,1): b_j[r] = sum_c A[r][c] * V[c][j] ---
    Astr = [A[:, c:c + 7:3] for c in range(3)]

    def bcol(j):
        t0 = em.cdve(OP_AXPBY, Astr[0], Astr[1],
                     s0=Vc[j][:, 0:1], s1=Vc[j][:, 1:2], n=3)
        return em.stt(Astr[2], Vc[j][:, 2:3], t0, ALU.mult, ALU.add, 3)

    def bcol_gps(j):
        t0 = em.gtt(ALU.mult, Astr[0],
                    Vc[j][:, 0:1].broadcast_to([32, 3]), 3)
        t1 = em.gtt(ALU.mult, Astr[1],
                    Vc[j][:, 1:2].broadcast_to([32, 3]), 3)
        t01 = em.gtt(ALU.add, t0, t1, 3)
        t2 = em.gtt(ALU.mult, Astr[2],
                    Vc[j][:, 2:3].broadcast_to([32, 3]), 3)
        return em.gtt(ALU.add, t01, t2, 3)

    b0 = bcol(0)
    b1 = bcol_gps(1)

    def normalize(v3):
        sqv = em.tt(ALU.mult, v3, v3, 3)
        n_ = em.tt(ALU.add, sqv[:, 0:1], sqv[:, 1:2])
        n_ = em.tt(ALU.add, n_, sqv[:, 2:3])
        nc_ = em.ts(ALU.max, n_, 1e-30)
        inv = em.rsqrt(nc_)
        return em.ts(ALU.mult, v3, inv, 3)

    u1 = normalize(b0)
    p_ = em.tt(ALU.mult, u1, b1, 3)
    d_ = em.tt(ALU.add, p_[:, 0:1], p_[:, 1:2])
    d_ = em.tt(ALU.add, d_, p_[:, 2:3])
    dneg = em.ts(ALU.mult, d_, -1.0)
    b2o = em.stt(u1, dneg, b1, ALU.mult, ALU.add, 3)
    u2 = normalize(b2o)
    u3 = em.new(3)
    for k, (i1, i2) in enumerate(((1, 2), (2, 0), (0, 1))):
        em.cdve(OP_AXMBY, u1[:, i1:i1 + 1], u1[:, i2:i2 + 1],
                s0=u2[:, i2:i2 + 1], s1=u2[:, i1:i1 + 1],
                out=u3[:, k:k + 1])

    # --- R = u1 v1^T + u2 v2^T + u3 v3^T ---
    t0 = em.tt3(ALU.mult, _bcast_r(u1), _bcast_l(Vc[0]), 9)
    t1 = em.tt3(ALU.mult, _bcast_r(u2), _bcast_l(Vc[1]), 9)
    t01 = em.tt(ALU.add, t0, t1, 9)
    t2 = em.tt3(ALU.mult, _bcast_r(u3), _bcast_l(Vc[2]), 9)
    pose_R = pose_tile[:].rearrange("p (r c) -> p r c", r=4, c=4)[:, 0:3, 0:3]
    nc.vector.tensor_tensor(
        pose_R, t01.rearrange("p (r c) -> p r c", r=3, c=3),
        t2.rearrange("p (r c) -> p r c", r=3, c=3), ALU.add)


# ---------------------------------------------------------------------------
# kernel build
# ---------------------------------------------------------------------------
def build_nc():
    nc = bacc.Bacc("TRN2", target_bir_lowering=False)

    # quarter-major x layout: [quarter, feature, token-in-quarter] so every
    # (chunk, quarter) DMA is one fully contiguous 256KB block
    xT8 = nc.dram_tensor("xT8", [4, D, 2048], F8, kind="ExternalInput")
    w8 = nc.dram_tensor("w8", [6, 128, 2048], F8, kind="ExternalInput")
    bs = nc.dram_tensor("bs", [6, D], F32, kind="ExternalInput")
    x0s = nc.dram_tensor("x0s", [128, 4 * S_CORE], F32, kind="ExternalInput")
    mwt = nc.dram_tensor("mwt", [2, D, D], BF16, kind="ExternalInput")
    mbs = nc.dram_tensor("mbs", [2, D], F32, kind="ExternalInput")
    hwT = nc.dram_tensor("hwT", [D, 12], BF16, kind="ExternalInput")
    hb = nc.dram_tensor("hb", [S_CORE, 12], F32, kind="ExternalInput")
    pose = nc.dram_tensor("pose", [S_CORE, 16], F32, kind="ExternalOutput")

    with tile.TileContext(nc) as tc:
        with (
            tc.tile_pool(name="wp", bufs=1) as wpool,
            tc.tile_pool(name="ps", bufs=4, space="PSUM") as pspool,
            tc.tile_pool(name="sm", bufs=1) as smpool,
        ):
            # warm both ACT tables (Sqrt for the SVD tail, Relu for drains)
            # while the first DMAs stream
            warm = smpool.tile([32, 1], F32, tag="warm", name="warm")
            nc.vector.memset(warm[:], 0.0)
            nc.scalar.activation(warm[:], warm[:], AF.Sqrt)
            nc.scalar.activation(warm[:], warm[:], AF.Relu)

            # ---- static activation buffers (feature-major: chunk k holds
            # features 128k..128k+127 for all 8192 tokens) ----
            X = wpool.tile([128, 4 * T_CORE], F8, tag="X", name="X")
            HA = wpool.tile([128, 4 * T_CORE], F8, tag="HA", name="HA")
            HB = wpool.tile([128, 4 * T_CORE], F8, tag="HB", name="HB")
            HC = wpool.tile([128, 4 * T_CORE], F8, tag="HC", name="HC")

            w_sb = [wpool.tile([128, 2048], F8, tag=f"w{l}", name=f"w{l}")
                    for l in range(6)]
            b_sb = wpool.tile([128, 24], F32, tag="b", name="b_sb")

            # X in chunk-quarters (the l0/ti(2q,2q+1) matmuls need quarter q
            # of every chunk), interleaved with the layer-0 weights so the
            # first matmuls fire a few us in and never starve after that
            def xdma(eng, c, q):
                eng.dma_start(
                    X[:, T_CORE * c + 2048 * q:T_CORE * c + 2048 * (q + 1)],
                    xT8[q, 128 * c:128 * (c + 1), :])

            def xdma8(eng, c, e):
                eng.dma_start(
                    X[:, T_CORE * c + 1024 * e:T_CORE * c + 1024 * (e + 1)],
                    xT8[0, 128 * c:128 * (c + 1),
                        1024 * e:1024 * (e + 1)])

            # first supertile as eighths so the very first matmuls fire ASAP
            xdma8(nc.gpsimd, 0, 0)
            xdma8(nc.scalar, 1, 0)
            nc.sync.dma_start(w_sb[0][:, 0:512], w8[0, :, 0:512])
            xdma8(nc.gpsimd, 2, 0)
            xdma8(nc.scalar, 3, 0)
            xdma8(nc.gpsimd, 0, 1)
            xdma8(nc.scalar, 1, 1)
            nc.sync.dma_start(w_sb[0][:, 512:2048], w8[0, :, 512:2048])
            xdma8(nc.gpsimd, 2, 1)
            xdma8(nc.scalar, 3, 1)
            # biases immediately behind w0: the first l0 drains need them
            for l in range(6):
                nc.sync.dma_start(b_sb[:, 4 * l:4 * l + 4],
                                  bs[l].rearrange("(o p) -> p o", p=128, o=4))
            # q1 rides partly on the sync queue (w1/w2 aren't needed for
            # ~30/60us) so the l0 stream never starves
            xdma(nc.sync, 0, 1)
            xdma(nc.scalar, 1, 1)
            xdma(nc.sync, 2, 1)
            xdma(nc.scalar, 3, 1)
            for q in range(2, 4):
                xdma(nc.gpsimd, 0, q)
                xdma(nc.gpsimd, 2, q)
            nc.sync.dma_start(w_sb[1][:], w8[1])
            nc.sync.dma_start(w_sb[2][:], w8[2])
            nc.sync.dma_start(w_sb[3][:], w8[3])
            nc.sync.dma_start(w_sb[5][:], w8[5])

            x0s_sb = wpool.tile([128, 4 * S_CORE], F32, tag="x0s",
                                name="x0s_sb")
            nc.sync.dma_start(x0s_sb[:], x0s[:])
            mw_sb = [wpool.tile([128, 2048], BF16, tag=f"mw{l}",
                                name=f"mw{l}") for l in range(2)]
            for l in range(2):
                for k in range(4):
                    nc.sync.dma_start(
                        mw_sb[l][:, D * k:D * (k + 1)],
                        mwt[l, 128 * k:128 * (k + 1), :])
            mb_sb = wpool.tile([128, 8], F32, tag="mb", name="mb_sb")
            for l in range(2):
                nc.sync.dma_start(mb_sb[:, 4 * l:4 * l + 4],
                                  mbs[l].rearrange("(o p) -> p o", p=128, o=4))
            hw_sb = wpool.tile([128, 48], BF16, tag="hw", name="hw_sb")
            for k in range(4):
                nc.sync.dma_start(hw_sb[:, 12 * k:12 * (k + 1)],
                                  hwT[128 * k:128 * (k + 1), :])
            hb_sb = wpool.tile([32, 12], F32, tag="hbt", name="hb_sb")
            nc.sync.dma_start(hb_sb[:], hb[:])

            # pooling accumulators: pb1 = per-sample Sum(h3a), pb2 =
            # per-sample Sum(h3b); both via DVE tensor_reduce (pb1 deferred
            # per-supertile, pb2 fine-grained right behind the l5 drains)
            pb1 = wpool.tile([128, 4 * S_CORE], F16, tag="pb1", name="pb1")
            pb2 = wpool.tile([128, 4 * S_CORE], F16, tag="pb2", name="pb2")
            # pooling tree scratch: tensor_reduce is stuck at 1 elem/cycle on
            # DVE, but 2-byte tensor_tensor hits the 2x packed mode - so sum
            # 256->128->64 with TT adds, then one small reduce
            tr1a = [wpool.tile([128, 2048], BF16, tag=f"tr1a{i}",
                               name=f"tr1a{i}") for i in range(2)]
            tr2a = wpool.tile([128, 1024], BF16, tag="tr2a", name="tr2a")
            tr1b = [wpool.tile([128, 2048], BF16, tag=f"tr1b{i}",
                               name=f"tr1b{i}") for i in range(2)]
            tr2b = [wpool.tile([128, 1024], BF16, tag=f"tr2b{i}",
                               name=f"tr2b{i}") for i in range(2)]

            def wap(l, o, kp):
                c0 = (o * 2 + kp) * 256
                return w_sb[l][:, c0:c0 + 256].rearrange(
                    "p (i m) -> p i m", i=2)

            def rhs(src, kp, ti, th):
                v = src[:].rearrange("p (k t) -> p k t", k=4)
                t0 = 1024 * ti + 512 * th
                return v[:, 2 * kp:2 * kp + 2, t0:t0 + 512]

            def hslice(dst, o, ti):
                c0 = T_CORE * o + 1024 * ti
                return dst[:, c0:c0 + 1024]

            def _src_v(buf, ti):
                return buf[:].rearrange("p (k g t) -> p k g t", k=4,
                                        g=S_CORE)[:, :, 4 * ti:4 * ti + 4, :]

            def pool_tree1(ti):
                """pb1 tree: level 1 split GPSIMD/DVE, level 2 + reduce DVE.
                Lives in the l3 phase, where all drains go to ACT, so these
                are the only DVE ops and never block a drain."""
                v = _src_v(HC, ti)
                d1 = tr1a[ti % 2][:].rearrange("p (k s t) -> p k s t",
                                               k=4, s=4)
                nc.gpsimd.tensor_tensor(
                    d1[:, 0:2], v[:, 0:2, :, 0:128], v[:, 0:2, :, 128:256],
                    ALU.add)
                nc.vector.tensor_tensor(
                    d1[:, 2:4], v[:, 2:4, :, 0:128], v[:, 2:4, :, 128:256],
                    ALU.add)
                t1 = tr1a[ti % 2][:].rearrange("p (g h t) -> p g h t",
                                               g=16, h=2)
                d2 = tr2a[:].rearrange("p (g t) -> p g t", g=16)
                nc.vector.tensor_tensor(d2, t1[:, :, 0, :], t1[:, :, 1, :],
                                        ALU.add)
                dst = pb1[:].rearrange("p (k s) -> p k s",
                                       k=4)[:, :, 4 * ti:4 * ti + 4]
                with nc.allow_low_precision("fp16 pooling partials"):
                    nc.vector.tensor_reduce(dst, d2, axis=AX.X, op=ALU.add)

            def pool_tree2(ti):
                """pb2 tree: level 1 split GPSIMD/DVE, level 2 + reduce DVE.
                Ping-pong scratch so consecutive supertiles' trees overlap
                (matters most for ti6/ti7 right at the end)."""
                v = _src_v(HB, ti)
                w1, w2 = tr1b[ti % 2], tr2b[ti % 2]
                d1 = w1[:].rearrange("p (k s t) -> p k s t", k=4, s=4)
                nc.gpsimd.tensor_tensor(
                    d1[:, 0:2], v[:, 0:2, :, 0:128], v[:, 0:2, :, 128:256],
                    ALU.add)
                nc.vector.tensor_tensor(
                    d1[:, 2:4], v[:, 2:4, :, 0:128], v[:, 2:4, :, 128:256],
                    ALU.add)
                t1 = w1[:].rearrange("p (g h t) -> p g h t", g=16, h=2)
                d2 = w2[:].rearrange("p (g t) -> p g t", g=16)
                nc.vector.tensor_tensor(d2, t1[:, :, 0, :], t1[:, :, 1, :],
                                        ALU.add)
                dst = pb2[:].rearrange("p (k s) -> p k s",
                                       k=4)[:, :, 4 * ti:4 * ti + 4]
                with nc.allow_low_precision("fp16 pooling partials"):
                    nc.vector.tensor_reduce(dst, d2, axis=AX.X, op=ALU.add)

            def drain(l, o, ti, ps, dst, eng):
                h = hslice(dst, o, ti)
                bias = b_sb[:, 4 * l + o:4 * l + o + 1]
                if eng == 'act':
                    nc.scalar.activation(h, ps[:], AF.Relu, bias=bias,
                                         scale=1.0)
                else:
                    nc.vector.tensor_scalar(h, ps[:], bias, 0.0,
                                            ALU.add, ALU.max)

            def x2_emit(ti):
                """x2 = x0 + h3a (fp8), chunk k: GPSIMD takes 0/1, DVE 2/3."""
                for k in (3, 2, 0, 1):
                    c0 = T_CORE * k + 1024 * ti
                    dst = HA[:, c0:c0 + 1024]
                    eng = nc.gpsimd if (GPS_X2 and k < 2) else nc.vector
                    eng.tensor_tensor(dst, HC[:, c0:c0 + 1024],
                                      X[:, c0:c0 + 1024], ALU.add)

            # ACT drains o0-o2, DVE o3 — except l3, where ACT takes all 4 so
            # the DVE queue holds only pb1 tree ops (no drain ever queues
            # behind a pooling op there)
            def eng_for(l, o, ti):
                if l == 3:
                    return 'act'
                return 'dve' if o == 3 else 'act'

            # buffer roles per layer: src, dst
            ROLES = [(X, HA), (HA, HB), (HB, HC), (HA, HB), (HB, X),
                     (X, HB)]

            # l4/l5 interleave in ti-pairs so h3b (and its pooling reduces)
            # starts ~40us before the end of the matmul stream
            pool_f32 = smpool.tile([128, 4 * S_CORE], F32, tag="poolf",
                                   name="pool_f32")
            pool_bf = smpool.tile([128, 4 * S_CORE], BF16, tag="poolb",
                                  name="pool_bf")

            PHASES = [(0, range(8)), (1, range(8)), (2, range(8)),
                      (3, range(8))]
            for p in range(4):
                PHASES.append((4, range(2 * p, 2 * p + 2)))
                PHASES.append((5, range(2 * p, 2 * p + 2)))

            # pb1 pooling placement: one tree per ti in the l3 phase
            PB1_AT = {(3, ti): ti for ti in range(N_SUP)}

            for pi, (l, tis) in enumerate(PHASES):
                src, dst = ROLES[l]
                for ti in tis:
                    for o in range(4):
                        pst = pspool.tile([128, 1024], F32, tag="ps",
                                          name="ps")
                        mi = 0
                        for kp in range(2):
                            for th in range(2):
                                nc.tensor.matmul(
                                    pst[:, 512 * th:512 * (th + 1)],
                                    wap(l, o, kp), rhs(src, kp, ti, th),
                                    start=(mi < 2), stop=(mi >= 2),
                                    perf_mode=DR)
                                mi += 1
                        drain(l, o, ti, pst, dst, eng_for(l, o, ti))
                    # scalar-queue X/w4 issues ride between early l0 drains
                    # (issuing them all in the preamble delays the first
                    # drains by ~6us of queue-issue time)
                    if l == 0 and ti in (1, 3, 5):
                        if ti == 5:
                            nc.scalar.dma_start(w_sb[4][:], w8[4])
                        else:
                            q = 2 if ti == 1 else 3
                            xdma(nc.scalar, 1, q)
                            xdma(nc.scalar, 3, q)
                    if l == 5:
                        pool_tree2(ti)
                    if l == 2:
                        x2_emit(ti)
                    t1_ = PB1_AT.get((pi, ti))
                    if t1_ is not None:
                        pool_tree1(t1_)
                    if pi == 10 and ti == 6:
                        # pb1 is long complete: pre-add x0s on idle GPSIMD so
                        # the tail needs only one combine op after pb2
                        nc.gpsimd.tensor_tensor(pool_f32[:], pb1[:],
                                                x0s_sb[:], ALU.add)

            # ---- pooled = (x0s + pb1) + pb2: single combine on DVE,
            # in-FIFO right behind the final reduce ----
            nc.vector.tensor_tensor(pool_bf[:], pool_f32[:], pb2[:],
                                    ALU.add)

            # ---- tail MLPs (bf16), psum reused from the main pool ----
            f_prev = pool_bf
            scales = [1.0 / TOK, 1.0]
            for l in range(2):
                f_out = smpool.tile([128, 4 * S_CORE], BF16, tag=f"f{l}",
                                    name=f"f{l}")
                for o in range(4):
                    ps_w = pspool.tile([128, 1024], F32, tag="ps",
                                       name="pst")
                    psm = ps_w[:, 0:S_CORE]
                    for k in range(4):
                        nc.tensor.matmul(
                            psm,
                            mw_sb[l][:, D * k + 128 * o:D * k + 128 * (o + 1)],
                            f_prev[:, S_CORE * k:S_CORE * (k + 1)],
                            start=(k == 0), stop=(k == 3))
                    nc.scalar.activation(
                        f_out[:, S_CORE * o:S_CORE * (o + 1)], psm, AF.Relu,
                        bias=mb_sb[:, 4 * l + o:4 * l + o + 1],
                        scale=scales[l])
                f_prev = f_out

            # ---- heads: [32 samples, 12] = t(3) ++ rot(9) ----
            psh_w = pspool.tile([128, 1024], F32, tag="ps", name="psh")
            psh = psh_w[0:32, 0:12]
            for k in range(4):
                nc.tensor.matmul(psh,
                                 f_prev[:, S_CORE * k:S_CORE * (k + 1)],
                                 hw_sb[:, 12 * k:12 * (k + 1)],
                                 start=(k == 0), stop=(k == 3))
            mm = smpool.tile([32, 12], F32, tag="mm", name="mm")
            nc.vector.tensor_add(mm[:], psh, hb_sb[:])

            # ---- pose assembly + SVD ----
            pose_t = smpool.tile([32, 16], F32, tag="pose", name="pose_t")
            nc.vector.memset(pose_t[:], 0.0)
            nc.vector.memset(pose_t[:, 15:16], 1.0)
            nc.vector.tensor_copy(
                pose_t[:].rearrange("p (r c) -> p r c", r=4, c=4)[:, 0:3, 3],
                mm[:, 0:3])

            em = Emit(nc, smpool)
            emit_svd_so3(nc, em, mm[:, 3:12], pose_t)

            nc.sync.dma_start(pose[:], pose_t[:])

    nc.compile()
    return nc


_NC_CACHE = None


def _get_nc():
    global _NC_CACHE
    if _NC_CACHE is None:
        _NC_CACHE = build_nc()
    return _NC_CACHE


F8NP = ml_dtypes.float8_e4m3fn
BF16NP = ml_dtypes.bfloat16


def kernel(**inputs):
    feat = np.asarray(inputs["feat"], dtype=np.float32)
    b_, v_, n_, d_ = feat.shape
    xs = feat.reshape(b_ * v_, n_, d_)
    x0sum = xs.sum(axis=1, dtype=np.float32)          # (256, 512)

    # DoubleRow weight prepack: [p, o, kp, i, m] <- wT[128*(2kp+i)+p, 128o+m]
    w8_list = []
    for blk in (1, 2):
        for li in (1, 2, 3):
            wT = np.asarray(inputs[f"r{blk}_w{li}"], np.float32).T
            arr = wT.astype(F8NP).reshape(2, 2, 128, 4, 128)
            arr = np.ascontiguousarray(arr.transpose(2, 3, 0, 1, 4))
            w8_list.append(arr.reshape(128, 2048))
    w8 = np.stack(w8_list)
    bs = np.stack([np.asarray(inputs[f"r{blk}_b{li}"], np.float32)
                   for blk in (1, 2) for li in (1, 2, 3)])
    mwt = np.stack([np.ascontiguousarray(
        np.asarray(inputs[f"m_w{li}"], np.float32).T).astype(BF16NP)
        for li in (1, 2)])
    mbs = np.stack([np.asarray(inputs[f"m_b{li}"], np.float32)
                    for li in (1, 2)])
    hwT = np.ascontiguousarray(np.concatenate(
        [np.asarray(inputs["t_w"], np.float32).T,
         np.asarray(inputs["rot_w"], np.float32).T], axis=1)).astype(BF16NP)
    hb = np.broadcast_to(np.concatenate(
        [np.asarray(inputs["t_b"], np.float32),
         np.asarray(inputs["rot_b"], np.float32)])[None, :],
        (S_CORE, 12)).copy()

    in_maps = []
    for c in range(N_CORES):
        xT = xs[c * S_CORE:(c + 1) * S_CORE].reshape(T_CORE, D).T  # (512, T)
        xT8 = np.ascontiguousarray(
            xT.reshape(D, 4, 2048).transpose(1, 0, 2)).astype(F8NP)
        xs_c = x0sum[c * S_CORE:(c + 1) * S_CORE]     # (32, 512)
        x0s = np.ascontiguousarray(
            xs_c.T.reshape(4, 128, S_CORE).transpose(1, 0, 2).reshape(
                128, 4 * S_CORE))
        in_maps.append({
            "xT8": xT8, "w8": w8, "bs": bs, "x0s": x0s, "mwt": mwt,
            "mbs": mbs, "hwT": hwT, "hb": hb,
        })

    nc = _get_nc()
    import os
    kwargs = {}
    if os.environ.get("KERNEL_TRACE") == "1":
        kwargs["trace"] = True
    res = run_bass_kernel_spmd(nc, in_maps, core_ids=list(range(N_CORES)),
                               **kwargs)
    if kwargs.get("trace"):
        kernel.last_results = res
    poses = np.concatenate([r["pose"] for r in res.results], axis=0)
    return poses.reshape(b_, v_, 4, 4)
